# revision 20
# baseline (speedup 1.0000x reference)
"""Bilateral filter (5x5, sigmaXY=sigmaZ=1) on 8 Trainium2 NeuronCores.

Math (p neighbor, c center, both in [0,1)), x := p-1/2, u := c-1/2:
    sim(p,c) = w_spatial * exp(-0.5(p-c)^2)
             = w_spatial * t0(p) * t0(c)e^{-1/8} * e^{xu},  e^{xu} ~= 1 + xu
    t_k = t0 * x^k,  t0 = e^{-x^2/2 + 1/8},  S_k = gauss5x5 (*) t_k
    den = S0 + u*S1,  M = S1 + u*S2,  out = 1/2 + M/den

Implementation notes:
  - x is centered (p - 1/2) on the host during fp16 layout prep.
  - All three convs are fp8e4 DoubleRow band-matmuls (2-ktile weights:
    e4m3 band + e4m3 residual -> ~fp11 kernel precision).
  - t1 = x*poly(x^2) is one custom DVE op (deg-2 minimax poly of
    e^{-u/2+1/8}, err ~1e-5); t2 = t1*x on Pool; t0 via ACT Square+Exp.
  - S1,S2 evacuate as ONE ACT copy from a single 4-bank PSUM tile;
    qd,qm compute as ONE broadcast TT; rden = custom 1/(ps0+qd) fused
    seed+Newton DVE op reading S0 directly from PSUM (no S0 evac).
  - Group-flat "404" layout everywhere in asm (junk rows at img-local
    rows >= 64 are dropped on the host after gather).

Engine split per 104-col chunk: ACT sq,t0,evac | DVE t1,qdm,rden,outm |
Pool t2,em | PE 30 DR matmuls.
"""

import numpy as np
from contextlib import ExitStack

import concourse.bass as bass
import concourse.bacc as bacc
import concourse.tile as tile
from concourse import mybir
from concourse.bass import AP
from concourse.bass_utils import run_bass_kernel_spmd
import ml_dtypes

import concourse.dve_ops as dvo
from concourse.dve_spec import Spec, Src0, Src1, C0, C1, C2, One, sq as dsq
from concourse.dve_spec import lower as dve_lower
from concourse.dve_uop import DveOpSpec
from concourse.dve_table_gen import dve_ver_for
from concourse.dve_ops import DveOp

F32 = mybir.dt.float32
F16 = mybir.dt.float16
F8 = mybir.dt.float8e4
NP_F16 = np.float16
NP_F8 = ml_dtypes.float8_e4m3
AOP = mybir.AluOpType
AF = mybir.ActivationFunctionType

N_CORES = 8
NIMG = 12            # 4 batch * 3 channels
H = 512
W = 512
ROWS = 64            # output rows per core
R = ROWS + 4         # input rows per core incl halo
WPAD = 524           # 512 + 2+2 conv pad + 8 slack for 5*104 chunking
NCHUNK = 5
CH_OUT = 104         # output cols per chunk
CH_IN = CH_OUT + 4   # input cols per chunk
M8 = 112             # fp8 stationary col count (16-aligned), 104 useful
GRP = 6              # imgs per matmul group (contiguous flat moving)
NMOV = GRP * R - 4   # 404: moving rows per matmul (img-flat, junk tails)

DEGREE = 1           # kept for test.py compat (cache key)

# ---- custom DVE op registration -------------------------------------------

# deg-2 minimax poly of f(u) = e^{-u/2 + 1/8} on u in [0, 0.25] (err ~1.1e-5)
_P0, _P1, _P2 = 1.13313755, -0.5657856, 0.13311594

# Newton seed for 1/den, den in [DEN_A, DEN_B]; rden = y0*(2 - den*y0),
# y0 = NA - NB*den. True minimax linear seed for relerr of 1 - x*y0.
DEN_A, DEN_B = 4.0, 9.2
_NB = 2.0 / (DEN_A * DEN_B + (DEN_A + DEN_B) ** 2 / 4.0)
_NA = (DEN_A + DEN_B) * _NB


def _register_op(name, body, reference, rd1):
    for op in dvo.OPS:
        if op.name == name:
            return op
    row = max(dvo._SUB_OPCODE_FOR_NAME.values()) + 1
    assert row < 0x20, "custom DVE opcode rows exhausted"
    spec = Spec(body=body, reference=reference)
    ver = dve_ver_for("TRN2")
    uops = dve_lower(spec, ver=ver)
    s = DveOpSpec(name=name, opcode=row, uops=uops, rd1_en=rd1)
    op = DveOp(name, spec, subdim=False, uops_sha={ver: s.sha(ver)})
    dvo.OPS.append(op)
    dvo.CUSTOM_DVE_SPECS[name] = spec
    dvo._SUB_OPCODE_FOR_NAME[name] = row
    return op


def _t1_ref(in0, in1, c0, c1, c2):
    x = in0.astype(np.float32)
    u = np.square(x)
    return x * (c0 + u * (c1 + u * c2))


_u = dsq(Src0)
T1C = _register_op("BILAT_T1_ANT", Src0 * (C0 + _u * (C1 + _u * C2)), _t1_ref,
                   rd1=False)


def _rden_ref(in0, in1, c0, c1, c2):
    x = in0.astype(np.float32) + in1.astype(np.float32)
    y0 = c0 - c1 * x
    return y0 * (2.0 - x * y0)


_x = Src0 + Src1
_y0 = C0 - C1 * _x
_e = _y0 * (One - _x * _y0)
RDEN = _register_op("BILAT_RDEN_ANT", _e + _y0, _rden_ref, rd1=True)

# ---- weights ---------------------------------------------------------------

_W1D = np.exp(-0.5 * np.array([4.0, 1.0, 0.0, 1.0, 4.0], dtype=np.float64)).astype(
    np.float32
)


def _e4m3(a):
    return np.asarray(a, np.float32).astype(NP_F8).astype(np.float32)


def _build_bands8() -> np.ndarray:
    """b8[q, dy, kt, o]: e4m3 band + e4m3 residual in the second k-tile."""
    b = np.zeros((CH_IN, 5, 2, M8), dtype=np.float32)
    for o in range(CH_OUT):
        for d in range(5):
            for dy in range(5):
                w = np.float32(_W1D[d] * _W1D[dy])
                w0 = _e4m3(w)
                b[o + d, dy, 0, o] = w0
                b[o + d, dy, 1, o] = _e4m3(w - w0)
    return b.astype(NP_F8)


# engine-assignment tunables
T2_ON_POOL = True    # t2 = t1*x on Pool (else DVE)
EM_ON_POOL = True    # em = s1e + qm on Pool (else DVE)
SQ_ON_ACT = True     # sq via ACT Square (else DVE TT)
WARM_MM = 18         # PE p-state warmup matmuls before real work


def build_nc(degree: int = DEGREE, bench_iters: int = 1):
    nc = bacc.Bacc("TRN2", target_bir_lowering=False)
    const_tensors = []
    for v in (0.125, 0.0):
        t_ = nc.alloc_sbuf_tensor(f"const-f32-{v}", [128, 1], F32)
        nc.const_aps.aps[(F32, v)] = t_.ap()
        const_tensors.append((t_, v))
    x_d = nc.dram_tensor("x", [WPAD, NIMG, R], F16, kind="ExternalInput")
    b8_d = nc.dram_tensor("b8", [CH_IN, 5, 2, M8], F8, kind="ExternalInput")
    y_d = nc.dram_tensor("y", [WPAD, 2, NMOV], F16, kind="ExternalOutput")

    with ExitStack() as ctx:
        tc = ctx.enter_context(tile.TileContext(nc))
        singles = ctx.enter_context(tc.tile_pool(name="singles", bufs=1))
        fields = ctx.enter_context(tc.tile_pool(name="fields", bufs=3))
        asm = ctx.enter_context(tc.tile_pool(name="asm", bufs=3))
        psum = ctx.enter_context(tc.tile_pool(name="psum", bufs=1, space="PSUM"))

        for t_, v in const_tensors:
            nc.gpsimd.memset(t_.ap(), v)
        b8 = singles.tile([CH_IN, 5, 2, M8], F8)
        # dummy activation: hoists the ACT table load into the DMA window
        warm = singles.tile([128, 1], F16)
        nc.scalar.activation(out=warm, in_=const_tensors[0][0].ap(),
                             func=AF.Exp, bias=0.125, scale=-0.5)
        # PE p-state warmup: the tensor engine ramps to full clock only after
        # ~3us of continuous execution; burn idle pre-DMA time on dummy
        # matmuls over memset data so real convs run at full speed.
        wmov = singles.tile([CH_IN, NIMG, R], F8)
        wst = singles.tile([CH_IN, 2, M8], F8)
        nc.gpsimd.memset(wmov[:], 0.0)
        nc.gpsimd.memset(wst[:], 0.0)

        def mov8(t, g, dy):
            """[108, 2(step 0), 404] moving AP into field tile t at group g, dy."""
            full = t[:]
            ap0 = [list(d) for d in full.ap][0]
            off = full.offset + g * (GRP * R) + dy
            return AP(full.tensor, off, [ap0, [0, 2], [1, NMOV]])

        def xc_src(j):
            """DRAM view [104, 2(g), 404] of centered x for chunk j's centers."""
            c0 = CH_OUT * j
            sl = x_d[c0 + 2 : c0 + 2 + CH_OUT]
            pdim = [list(d) for d in sl.ap][0]
            return AP(sl.tensor, sl.offset + 2,
                      [pdim, [GRP * R, 2], [1, NMOV]])

        def bc2(t):
            """[104, 2(bcast), 2(g), 404] view of a [104, 2, 404] tile."""
            full = t[:]
            pdim = [list(d) for d in full.ap][0]
            return AP(full.tensor, full.offset,
                      [pdim, [0, 2], [NMOV, 2], [1, NMOV]])

        def body():
            xts, xcs, flds = {}, {}, {}

            def load_x(j):
                c0 = CH_OUT * j
                x_t = fields.tile([CH_IN, NIMG, R], F16, name="x_t", tag="x_t",
                                  bufs=NCHUNK)
                nc.sync.dma_start(out=x_t, in_=x_d[c0 : c0 + CH_IN])
                xts[j] = x_t

            def load_xc(j):
                x_cg = fields.tile([CH_OUT, 2, NMOV], F16, name="x_cg",
                                   tag="x_cg", bufs=NCHUNK)
                nc.sync.dma_start(out=x_cg, in_=xc_src(j))
                xcs[j] = x_cg

            def load_fields(j):
                x_t = xts[j]
                sqt = fields.tile([CH_IN, NIMG, R], F16, name="sq", tag="sq",
                                  bufs=2)
                if SQ_ON_ACT:
                    nc.scalar.activation(out=sqt, in_=x_t, func=AF.Square,
                                         bias=0.0, scale=1.0)
                else:
                    nc.vector.tensor_mul(sqt, x_t, x_t)
                t0 = fields.tile([CH_IN, NIMG, R], F8, name="t0", tag="t0",
                                 bufs=2)
                nc.scalar.activation(out=t0, in_=sqt, func=AF.Exp,
                                     bias=0.125, scale=-0.5)
                t1 = fields.tile([CH_IN, NIMG, R], F8, name="t1", tag="t1",
                                 bufs=2)
                nc.vector._custom_dve(T1C, out=t1[:], in0=x_t[:],
                                      s0=_P0, s1=_P1, imm2=_P2)
                t2 = fields.tile([CH_IN, NIMG, R], F8, name="t2", tag="t2",
                                 bufs=2)
                eng = nc.gpsimd if T2_ON_POOL else nc.vector
                eng.tensor_mul(t2, t1, x_t)
                flds[j] = (t0, t1, t2)

            def conv(ps_slice, tk, g):
                for dy in range(5):
                    nc.tensor.matmul(
                        ps_slice,
                        b8[:, dy, :, :],
                        mov8(tk, g, dy),
                        start=(dy == 0),
                        stop=(dy == 4),
                        perf_mode=mybir.MatmulPerfMode.DoubleRow,
                    )

            def warmup(n_mm):
                """Dummy DR matmuls to ramp PE; reuses ps2's bank (PE-serial)."""
                wps = psum.tile([M8, 2, 512], F32, name="wps", tag="ps2")
                for i in range(n_mm):
                    nc.tensor.matmul(
                        wps[:, 0, 0:NMOV], wst[:],
                        AP(wmov[:].tensor, wmov[:].offset,
                           [[list(d) for d in wmov[:].ap][0], [0, 2], [1, NMOV]]),
                        start=True, stop=True,
                        perf_mode=mybir.MatmulPerfMode.DoubleRow,
                    )

            def chunk_stage(j):
                """Convs + evac + asm for chunk j, ordered S1, S0, S2 to match
                field readiness (t1 DVE first, t0 ACT, t2 Pool last); evacs
                overlap later convs and free psum banks early."""
                t0, t1, t2 = flds[j]
                c0 = CH_OUT * j
                n_out = min(CH_OUT, W - c0)
                ps0 = psum.tile([M8, 2, 512], F32, name="ps0", tag="ps0",
                                bufs=2)
                ps1 = psum.tile([M8, 2, 512], F32, name="ps1", tag="ps1")
                ps2 = psum.tile([M8, 2, 512], F32, name="ps2", tag="ps2")
                s12e = asm.tile([CH_OUT, 2, 2, NMOV], F16, name="s12e",
                                tag="s12e")
                for g in range(2):
                    conv(ps1[:, g, 0:NMOV], t1, g)
                nc.scalar.copy(out=s12e[:, 0], in_=ps1[0:CH_OUT, :, 0:NMOV])
                # qd = x_c * S1 early, so rden can overlap the S2 matmuls
                qd = asm.tile([CH_OUT, 2, NMOV], F16, name="qd", tag="qd")
                nc.vector.tensor_mul(qd, xcs[j], s12e[:, 0])
                for g in range(2):
                    conv(ps0[:, g, 0:NMOV], t0, g)
                # rden = 1/(S0 + qd), S0 read straight from PSUM
                rden = asm.tile([CH_OUT, 2, NMOV], F16, name="rden", tag="rden")
                nc.vector._custom_dve(
                    RDEN, out=rden[:], in0=ps0[0:CH_OUT, :, 0:NMOV],
                    in1=qd[:], s0=_NA, s1=_NB,
                )
                for g in range(2):
                    conv(ps2[:, g, 0:NMOV], t2, g)
                nc.scalar.copy(out=s12e[:, 1], in_=ps2[0:CH_OUT, :, 0:NMOV])
                # qm = x_c * S2
                qm = asm.tile([CH_OUT, 2, NMOV], F16, name="qm", tag="qm")
                nc.vector.tensor_mul(qm, xcs[j], s12e[:, 1])
                # em = S1 + qm
                em = asm.tile([CH_OUT, 2, NMOV], F16, name="em", tag="em")
                eng = nc.gpsimd if EM_ON_POOL else nc.vector
                eng.tensor_add(em, s12e[:, 0], qm)
                if j + 2 < NCHUNK:
                    load_fields(j + 2)
                # outm = em * rden
                outm = asm.tile([CH_OUT, 2, NMOV], F16, name="outm", tag="outm")
                nc.vector.tensor_mul(outm, em, rden)
                nc.sync.dma_start(
                    out=y_d[c0 + 2 : c0 + 2 + n_out], in_=outm[:n_out]
                )

            warmup(WARM_MM)
            load_x(0)
            nc.sync.dma_start(out=b8, in_=b8_d[:])
            load_x(1)
            load_x(2)
            load_xc(0)
            load_x(3)
            load_xc(1)
            load_x(4)
            load_xc(2)
            load_xc(3)
            load_xc(4)
            load_fields(0)
            load_fields(1)
            for j in range(NCHUNK):
                chunk_stage(j)

        if bench_iters == 1:
            body()
        else:
            hints = (
                mybir.EngineType.PE,
                mybir.EngineType.DVE,
                mybir.EngineType.Activation,
                mybir.EngineType.SP,
            )
            with tc.For_i(0, bench_iters, 1, hint_engines=hints):
                body()

    nc.finalize()
    return nc


def _prep_inputs(X: np.ndarray):
    """Full X [4,3,512,512] fp32 -> per-core centered/transposed fp16 + weights."""
    Xr = np.ascontiguousarray(np.asarray(X, dtype=np.float32).reshape(NIMG, H, W))
    b8 = _build_bands8()
    in_maps = []
    for i in range(N_CORES):
        lo = ROWS * i - 2
        s0, s1 = max(0, lo), min(H, lo + R)
        # reference zero-pads in p-space; centered x = p - 1/2 -> pad is -0.5
        P = np.full((NIMG, R, WPAD), -0.5, dtype=np.float32)
        P[:, s0 - lo : s1 - lo, 2 : 2 + W] = Xr[:, s0:s1, :] - 0.5
        xt = np.ascontiguousarray(P.transpose(2, 0, 1)).astype(NP_F16)
        in_maps.append({"x": xt, "b8": b8})
    return in_maps


_NC_CACHE = {}

# host-side gather index: flat m -> (img, row) keeping img-local rows < 64
_MVALID = np.array([m for m in range(NMOV) if m % R < ROWS], dtype=np.int64)


def kernel(X: np.ndarray) -> np.ndarray:
    key = (DEGREE, 1)
    if key not in _NC_CACHE:
        _NC_CACHE[key] = build_nc(DEGREE, 1)
    nc = _NC_CACHE[key]
    in_maps = _prep_inputs(X)
    res = run_bass_kernel_spmd(nc, in_maps, list(range(N_CORES)))
    out = np.empty((NIMG, H, W), dtype=np.float32)
    for i in range(N_CORES):
        yi = np.asarray(res.results[i]["y"], dtype=np.float32)  # [WPAD, 2, NMOV]
        # [W, 2, 384] -> (g, img-in-grp, row) -> [12, 64, W]
        yv = yi[2 : 2 + W][:, :, _MVALID].reshape(W, 2, GRP, ROWS)
        out[:, ROWS * i : ROWS * (i + 1), :] = (
            0.5 + yv.reshape(W, NIMG, ROWS).transpose(1, 2, 0)
        )
    return out.reshape(4, 3, H, W)


# revision 25
# speedup vs baseline: 1.0577x; 1.0577x over previous
"""Bilateral filter (5x5, sigmaXY=sigmaZ=1) on 8 Trainium2 NeuronCores.

Math (p neighbor, c center, both in [0,1)), x := p-1/2, u := c-1/2:
    sim(p,c) = w_spatial * exp(-0.5(p-c)^2)
             = w_spatial * t0(p) * t0(c)e^{-1/8} * e^{xu},  e^{xu} ~= 1 + xu
    t_k = t0 * x^k,  t0 = e^{-x^2/2 + 1/8},  S_k = gauss5x5 (*) t_k
    den = S0 + u*S1,  M = S1 + u*S2,  out = 1/2 + M/den

Implementation notes:
  - x is centered (p - 1/2) on the host during fp16 layout prep.
  - All three convs are fp8e4 DoubleRow band-matmuls (2-ktile weights:
    e4m3 band + e4m3 residual -> ~fp11 kernel precision).
  - t1 = x*poly(x^2) is one custom DVE op (deg-2 minimax poly of
    e^{-u/2+1/8}, err ~1e-5); t2 = t1*x on Pool; t0 via ACT Square+Exp.
  - S1,S2 evacuate as ONE ACT copy from a single 4-bank PSUM tile;
    qd,qm compute as ONE broadcast TT; rden = custom 1/(ps0+qd) fused
    seed+Newton DVE op reading S0 directly from PSUM (no S0 evac).
  - Group-flat "404" layout everywhere in asm (junk rows at img-local
    rows >= 64 are dropped on the host after gather).

Engine split per 104-col chunk: ACT sq,t0,evac | DVE t1,qdm,rden,outm |
Pool t2,em | PE 30 DR matmuls.
"""

import numpy as np
from contextlib import ExitStack

import concourse.bass as bass
import concourse.bacc as bacc
import concourse.tile as tile
from concourse import mybir
from concourse.bass import AP
from concourse.bass_utils import run_bass_kernel_spmd
import ml_dtypes

import concourse.dve_ops as dvo
from concourse.dve_spec import Spec, Src0, Src1, C0, C1, C2, One, sq as dsq
from concourse.dve_spec import lower as dve_lower
from concourse.dve_uop import DveOpSpec
from concourse.dve_table_gen import dve_ver_for
from concourse.dve_ops import DveOp

F32 = mybir.dt.float32
F16 = mybir.dt.float16
F8 = mybir.dt.float8e4
NP_F16 = np.float16
NP_F8 = ml_dtypes.float8_e4m3
AOP = mybir.AluOpType
AF = mybir.ActivationFunctionType

N_CORES = 8
NIMG = 12            # 4 batch * 3 channels
H = 512
W = 512
ROWS = 64            # output rows per core
R = ROWS + 4         # input rows per core incl halo
WPAD = 524           # 512 + 2+2 conv pad + 8 slack for 5*104 chunking
NCHUNK = 5
CH_OUT = 104         # output cols per chunk
CH_IN = CH_OUT + 4   # input cols per chunk
M8 = 112             # fp8 stationary col count (16-aligned), 104 useful
GRP = 6              # imgs per matmul group (contiguous flat moving)
NMOV = GRP * R - 4   # 404: moving rows per matmul (img-flat, junk tails)

DEGREE = 1           # kept for test.py compat (cache key)

# ---- custom DVE op registration -------------------------------------------

# deg-2 minimax poly of f(u) = e^{-u/2 + 1/8} on u in [0, 0.25] (err ~1.1e-5)
_P0, _P1, _P2 = 1.13313755, -0.5657856, 0.13311594

# Newton seed for 1/den, den in [DEN_A, DEN_B]; rden = y0*(2 - den*y0),
# y0 = NA - NB*den. True minimax linear seed for relerr of 1 - x*y0.
DEN_A, DEN_B = 4.0, 9.2
_NB = 2.0 / (DEN_A * DEN_B + (DEN_A + DEN_B) ** 2 / 4.0)
_NA = (DEN_A + DEN_B) * _NB


def _register_op(name, body, reference, rd1):
    for op in dvo.OPS:
        if op.name == name:
            return op
    row = max(dvo._SUB_OPCODE_FOR_NAME.values()) + 1
    assert row < 0x20, "custom DVE opcode rows exhausted"
    spec = Spec(body=body, reference=reference)
    ver = dve_ver_for("TRN2")
    uops = dve_lower(spec, ver=ver)
    s = DveOpSpec(name=name, opcode=row, uops=uops, rd1_en=rd1)
    op = DveOp(name, spec, subdim=False, uops_sha={ver: s.sha(ver)})
    dvo.OPS.append(op)
    dvo.CUSTOM_DVE_SPECS[name] = spec
    dvo._SUB_OPCODE_FOR_NAME[name] = row
    return op


def _t1_ref(in0, in1, c0, c1, c2):
    x = in0.astype(np.float32)
    u = np.square(x)
    return x * (c0 + u * (c1 + u * c2))


_u = dsq(Src0)
T1C = _register_op("BILAT_T1_ANT", Src0 * (C0 + _u * (C1 + _u * C2)), _t1_ref,
                   rd1=False)


def _rden_ref(in0, in1, c0, c1, c2):
    x = in0.astype(np.float32) + in1.astype(np.float32)
    y0 = c0 - c1 * x
    return y0 * (2.0 - x * y0)


_x = Src0 + Src1
_y0 = C0 - C1 * _x
_e = _y0 * (One - _x * _y0)
RDEN = _register_op("BILAT_RDEN_ANT", _e + _y0, _rden_ref, rd1=True)

# ---- weights ---------------------------------------------------------------

_W1D = np.exp(-0.5 * np.array([4.0, 1.0, 0.0, 1.0, 4.0], dtype=np.float64)).astype(
    np.float32
)


def _e4m3(a):
    return np.asarray(a, np.float32).astype(NP_F8).astype(np.float32)


def _build_bands8() -> np.ndarray:
    """b8[q, dy, kt, o]: e4m3 band + e4m3 residual in the second k-tile."""
    b = np.zeros((CH_IN, 5, 2, M8), dtype=np.float32)
    for o in range(CH_OUT):
        for d in range(5):
            for dy in range(5):
                w = np.float32(_W1D[d] * _W1D[dy])
                w0 = _e4m3(w)
                b[o + d, dy, 0, o] = w0
                b[o + d, dy, 1, o] = _e4m3(w - w0)
    return b.astype(NP_F8)


# engine-assignment tunables
T2_ON_POOL = True    # t2 = t1*x on Pool (else DVE)
EM_ON_POOL = True    # em = s1e + qm on Pool (else DVE)
SQ_ON_ACT = True     # sq via ACT Square (else DVE TT)
WARM_MM = 15         # PE p-state warmup matmuls before real work


def build_nc(degree: int = DEGREE, bench_iters: int = 1):
    nc = bacc.Bacc("TRN2", target_bir_lowering=False)
    const_tensors = []
    for v in (0.125, 0.0):
        t_ = nc.alloc_sbuf_tensor(f"const-f32-{v}", [128, 1], F32)
        nc.const_aps.aps[(F32, v)] = t_.ap()
        const_tensors.append((t_, v))
    x_d = nc.dram_tensor("x", [WPAD, NIMG, R], F16, kind="ExternalInput")
    b8_d = nc.dram_tensor("b8", [CH_IN, 5, 2, M8], F8, kind="ExternalInput")
    y_d = nc.dram_tensor("y", [WPAD, 2, NMOV], F16, kind="ExternalOutput")

    with ExitStack() as ctx:
        tc = ctx.enter_context(tile.TileContext(nc))
        singles = ctx.enter_context(tc.tile_pool(name="singles", bufs=1))
        fields = ctx.enter_context(tc.tile_pool(name="fields", bufs=3))
        asm = ctx.enter_context(tc.tile_pool(name="asm", bufs=3))
        psum = ctx.enter_context(tc.tile_pool(name="psum", bufs=1, space="PSUM"))

        for t_, v in const_tensors:
            nc.gpsimd.memset(t_.ap(), v)
        b8 = singles.tile([CH_IN, 5, 2, M8], F8)
        # dummy activation: hoists the ACT table load into the DMA window
        warm = singles.tile([128, 1], F16)
        nc.scalar.activation(out=warm, in_=const_tensors[0][0].ap(),
                             func=AF.Exp, bias=0.125, scale=-0.5)
        # PE p-state warmup: the tensor engine ramps to full clock only after
        # ~3us of continuous execution; burn idle pre-DMA time on dummy
        # matmuls over memset data so real convs run at full speed.
        wmov = singles.tile([CH_IN, NIMG, R], F8)
        wst = singles.tile([CH_IN, 2, M8], F8)
        nc.gpsimd.memset(wmov[:], 0.0)
        nc.gpsimd.memset(wst[:], 0.0)

        def mov8(t, g, dy):
            """[108, 2(step 0), 404] moving AP into field tile t at group g, dy."""
            full = t[:]
            ap0 = [list(d) for d in full.ap][0]
            off = full.offset + g * (GRP * R) + dy
            return AP(full.tensor, off, [ap0, [0, 2], [1, NMOV]])

        def xc_src(j):
            """DRAM view [104, 2(g), 404] of centered x for chunk j's centers."""
            c0 = CH_OUT * j
            sl = x_d[c0 + 2 : c0 + 2 + CH_OUT]
            pdim = [list(d) for d in sl.ap][0]
            return AP(sl.tensor, sl.offset + 2,
                      [pdim, [GRP * R, 2], [1, NMOV]])

        def bc2(t):
            """[104, 2(bcast), 2(g), 404] view of a [104, 2, 404] tile."""
            full = t[:]
            pdim = [list(d) for d in full.ap][0]
            return AP(full.tensor, full.offset,
                      [pdim, [0, 2], [NMOV, 2], [1, NMOV]])

        def body():
            xts, xcs, flds = {}, {}, {}

            def load_x(j):
                c0 = CH_OUT * j
                x_t = fields.tile([CH_IN, NIMG, R], F16, name="x_t", tag="x_t",
                                  bufs=NCHUNK)
                nc.sync.dma_start(out=x_t, in_=x_d[c0 : c0 + CH_IN])
                xts[j] = x_t

            def load_xc(j):
                x_cg = fields.tile([CH_OUT, 2, NMOV], F16, name="x_cg",
                                   tag="x_cg", bufs=NCHUNK)
                nc.sync.dma_start(out=x_cg, in_=xc_src(j))
                xcs[j] = x_cg

            def load_fields(j):
                x_t = xts[j]
                sqt = fields.tile([CH_IN, NIMG, R], F16, name="sq", tag="sq",
                                  bufs=2)
                if SQ_ON_ACT:
                    nc.scalar.activation(out=sqt, in_=x_t, func=AF.Square,
                                         bias=0.0, scale=1.0)
                else:
                    nc.vector.tensor_mul(sqt, x_t, x_t)
                t0 = fields.tile([CH_IN, NIMG, R], F8, name="t0", tag="t0",
                                 bufs=2)
                nc.scalar.activation(out=t0, in_=sqt, func=AF.Exp,
                                     bias=0.125, scale=-0.5)
                t1 = fields.tile([CH_IN, NIMG, R], F8, name="t1", tag="t1",
                                 bufs=2)
                nc.vector._custom_dve(T1C, out=t1[:], in0=x_t[:],
                                      s0=_P0, s1=_P1, imm2=_P2)
                t2 = fields.tile([CH_IN, NIMG, R], F8, name="t2", tag="t2",
                                 bufs=2)
                eng = nc.gpsimd if T2_ON_POOL else nc.vector
                eng.tensor_mul(t2, t1, x_t)
                flds[j] = (t0, t1, t2)

            def conv(ps_slice, tk, g):
                for dy in range(5):
                    nc.tensor.matmul(
                        ps_slice,
                        b8[:, dy, :, :],
                        mov8(tk, g, dy),
                        start=(dy == 0),
                        stop=(dy == 4),
                        perf_mode=mybir.MatmulPerfMode.DoubleRow,
                    )

            def warmup(n_mm):
                """Dummy DR matmuls to ramp PE; borrows a ps02 buffer (PE-serial)."""
                wps = psum.tile([M8, 2, 512], F32, name="wps", tag="ps02",
                                bufs=2)
                for i in range(n_mm):
                    nc.tensor.matmul(
                        wps[:, 0, 0:NMOV], wst[:],
                        AP(wmov[:].tensor, wmov[:].offset,
                           [[list(d) for d in wmov[:].ap][0], [0, 2], [1, NMOV]]),
                        start=True, stop=True,
                        perf_mode=mybir.MatmulPerfMode.DoubleRow,
                    )

            def chunk_stage(j):
                """Convs + evac + asm for chunk j, ordered S1, S0, S2 to match
                field readiness (t1 DVE first, t0 ACT, t2 Pool last); evacs
                overlap later convs and free psum banks early."""
                t0, t1, t2 = flds[j]
                c0 = CH_OUT * j
                n_out = min(CH_OUT, W - c0)
                # ps1 double-buffered: S1(j+1) must not wait on the s1e evac.
                # ps0/ps2 share one double-buffered tag (alternating gens), so
                # each still gets a full chunk of slack. 4+4 = 8 banks.
                ps0 = psum.tile([M8, 2, 512], F32, name="ps0", tag="ps02",
                                bufs=2)
                ps1 = psum.tile([M8, 2, 512], F32, name="ps1", tag="ps1",
                                bufs=2)
                ps2 = psum.tile([M8, 2, 512], F32, name="ps2", tag="ps02",
                                bufs=2)
                s12e = asm.tile([CH_OUT, 2, 2, NMOV], F16, name="s12e",
                                tag="s12e")
                for g in range(2):
                    conv(ps1[:, g, 0:NMOV], t1, g)
                nc.scalar.copy(out=s12e[:, 0], in_=ps1[0:CH_OUT, :, 0:NMOV])
                # qd = x_c * S1 early, so rden can overlap the S2 matmuls
                qd = asm.tile([CH_OUT, 2, NMOV], F16, name="qd", tag="qd")
                nc.vector.tensor_mul(qd, xcs[j], s12e[:, 0])
                for g in range(2):
                    conv(ps0[:, g, 0:NMOV], t0, g)
                # rden = 1/(S0 + qd), S0 read straight from PSUM
                rden = asm.tile([CH_OUT, 2, NMOV], F16, name="rden", tag="rden")
                nc.vector._custom_dve(
                    RDEN, out=rden[:], in0=ps0[0:CH_OUT, :, 0:NMOV],
                    in1=qd[:], s0=_NA, s1=_NB,
                )
                for g in range(2):
                    conv(ps2[:, g, 0:NMOV], t2, g)
                nc.scalar.copy(out=s12e[:, 1], in_=ps2[0:CH_OUT, :, 0:NMOV])
                # qm = x_c * S2
                qm = asm.tile([CH_OUT, 2, NMOV], F16, name="qm", tag="qm")
                nc.vector.tensor_mul(qm, xcs[j], s12e[:, 1])
                # em = S1 + qm; DVE for the tail chunks (Pool's 1.7us ops
                # would serialize the drain after PE finishes)
                em = asm.tile([CH_OUT, 2, NMOV], F16, name="em", tag="em")
                eng = nc.gpsimd if (EM_ON_POOL and j < 3) else nc.vector
                eng.tensor_add(em, s12e[:, 0], qm)
                if j + 2 < NCHUNK:
                    load_fields(j + 2)
                # outm = em * rden
                outm = asm.tile([CH_OUT, 2, NMOV], F16, name="outm", tag="outm")
                nc.vector.tensor_mul(outm, em, rden)
                nc.sync.dma_start(
                    out=y_d[c0 + 2 : c0 + 2 + n_out], in_=outm[:n_out]
                )

            warmup(WARM_MM)
            load_x(0)
            nc.sync.dma_start(out=b8, in_=b8_d[:])
            load_x(1)
            load_x(2)
            load_xc(0)
            load_x(3)
            load_xc(1)
            load_x(4)
            load_xc(2)
            load_xc(3)
            load_xc(4)
            load_fields(0)
            load_fields(1)
            for j in range(NCHUNK):
                chunk_stage(j)

        if bench_iters == 1:
            body()
        else:
            hints = (
                mybir.EngineType.PE,
                mybir.EngineType.DVE,
                mybir.EngineType.Activation,
                mybir.EngineType.SP,
            )
            with tc.For_i(0, bench_iters, 1, hint_engines=hints):
                body()

    nc.finalize()
    return nc


def _prep_inputs(X: np.ndarray):
    """Full X [4,3,512,512] fp32 -> per-core centered/transposed fp16 + weights."""
    Xr = np.ascontiguousarray(np.asarray(X, dtype=np.float32).reshape(NIMG, H, W))
    b8 = _build_bands8()
    in_maps = []
    for i in range(N_CORES):
        lo = ROWS * i - 2
        s0, s1 = max(0, lo), min(H, lo + R)
        # reference zero-pads in p-space; centered x = p - 1/2 -> pad is -0.5
        P = np.full((NIMG, R, WPAD), -0.5, dtype=np.float32)
        P[:, s0 - lo : s1 - lo, 2 : 2 + W] = Xr[:, s0:s1, :] - 0.5
        xt = np.ascontiguousarray(P.transpose(2, 0, 1)).astype(NP_F16)
        in_maps.append({"x": xt, "b8": b8})
    return in_maps


_NC_CACHE = {}

# host-side gather index: flat m -> (img, row) keeping img-local rows < 64
_MVALID = np.array([m for m in range(NMOV) if m % R < ROWS], dtype=np.int64)


def kernel(X: np.ndarray) -> np.ndarray:
    key = (DEGREE, 1)
    if key not in _NC_CACHE:
        _NC_CACHE[key] = build_nc(DEGREE, 1)
    nc = _NC_CACHE[key]
    in_maps = _prep_inputs(X)
    res = run_bass_kernel_spmd(nc, in_maps, list(range(N_CORES)))
    out = np.empty((NIMG, H, W), dtype=np.float32)
    for i in range(N_CORES):
        yi = np.asarray(res.results[i]["y"], dtype=np.float32)  # [WPAD, 2, NMOV]
        # [W, 2, 384] -> (g, img-in-grp, row) -> [12, 64, W]
        yv = yi[2 : 2 + W][:, :, _MVALID].reshape(W, 2, GRP, ROWS)
        out[:, ROWS * i : ROWS * (i + 1), :] = (
            0.5 + yv.reshape(W, NIMG, ROWS).transpose(1, 2, 0)
        )
    return out.reshape(4, 3, H, W)


# revision 26
# speedup vs baseline: 1.0610x; 1.0031x over previous
"""Bilateral filter (5x5, sigmaXY=sigmaZ=1) on 8 Trainium2 NeuronCores.

Math (p neighbor, c center, both in [0,1)), x := p-1/2, u := c-1/2:
    sim(p,c) = w_spatial * exp(-0.5(p-c)^2)
             = w_spatial * t0(p) * t0(c)e^{-1/8} * e^{xu},  e^{xu} ~= 1 + xu
    t_k = t0 * x^k,  t0 = e^{-x^2/2 + 1/8},  S_k = gauss5x5 (*) t_k
    den = S0 + u*S1,  M = S1 + u*S2,  out = 1/2 + M/den

Implementation notes:
  - x is centered (p - 1/2) on the host during fp16 layout prep.
  - All three convs are fp8e4 DoubleRow band-matmuls (2-ktile weights:
    e4m3 band + e4m3 residual -> ~fp11 kernel precision).
  - t1 = x*poly(x^2) is one custom DVE op (deg-2 minimax poly of
    e^{-u/2+1/8}, err ~1e-5); t2 = t1*x on Pool; t0 via ACT Square+Exp.
  - S1,S2 evacuate as ONE ACT copy from a single 4-bank PSUM tile;
    qd,qm compute as ONE broadcast TT; rden = custom 1/(ps0+qd) fused
    seed+Newton DVE op reading S0 directly from PSUM (no S0 evac).
  - Group-flat "404" layout everywhere in asm (junk rows at img-local
    rows >= 64 are dropped on the host after gather).

Engine split per 104-col chunk: ACT sq,t0,evac | DVE t1,qdm,rden,outm |
Pool t2,em | PE 30 DR matmuls.
"""

import numpy as np
from contextlib import ExitStack

import concourse.bass as bass
import concourse.bacc as bacc
import concourse.tile as tile
from concourse import mybir
from concourse.bass import AP
from concourse.bass_utils import run_bass_kernel_spmd
import ml_dtypes

import concourse.dve_ops as dvo
from concourse.dve_spec import Spec, Src0, Src1, C0, C1, C2, One, sq as dsq
from concourse.dve_spec import lower as dve_lower
from concourse.dve_uop import DveOpSpec
from concourse.dve_table_gen import dve_ver_for
from concourse.dve_ops import DveOp

F32 = mybir.dt.float32
F16 = mybir.dt.float16
F8 = mybir.dt.float8e4
NP_F16 = np.float16
NP_F8 = ml_dtypes.float8_e4m3
AOP = mybir.AluOpType
AF = mybir.ActivationFunctionType

N_CORES = 8
NIMG = 12            # 4 batch * 3 channels
H = 512
W = 512
ROWS = 64            # output rows per core
R = ROWS + 4         # input rows per core incl halo
WPAD = 524           # 512 + 2+2 conv pad + 8 slack for 5*104 chunking
NCHUNK = 5
CH_OUT = 104         # output cols per chunk
CH_IN = CH_OUT + 4   # input cols per chunk
M8 = 112             # fp8 stationary col count (16-aligned), 104 useful
GRP = 6              # imgs per matmul group (contiguous flat moving)
NMOV = GRP * R - 4   # 404: moving rows per matmul (img-flat, junk tails)

DEGREE = 1           # kept for test.py compat (cache key)

# ---- custom DVE op registration -------------------------------------------

# deg-2 minimax poly of f(u) = e^{-u/2 + 1/8} on u in [0, 0.25] (err ~1.1e-5)
_P0, _P1, _P2 = 1.13313755, -0.5657856, 0.13311594

# Newton seed for 1/den, den in [DEN_A, DEN_B]; rden = y0*(2 - den*y0),
# y0 = NA - NB*den. True minimax linear seed for relerr of 1 - x*y0.
DEN_A, DEN_B = 4.0, 9.2
_NB = 2.0 / (DEN_A * DEN_B + (DEN_A + DEN_B) ** 2 / 4.0)
_NA = (DEN_A + DEN_B) * _NB


def _register_op(name, body, reference, rd1):
    for op in dvo.OPS:
        if op.name == name:
            return op
    row = max(dvo._SUB_OPCODE_FOR_NAME.values()) + 1
    assert row < 0x20, "custom DVE opcode rows exhausted"
    spec = Spec(body=body, reference=reference)
    ver = dve_ver_for("TRN2")
    uops = dve_lower(spec, ver=ver)
    s = DveOpSpec(name=name, opcode=row, uops=uops, rd1_en=rd1)
    op = DveOp(name, spec, subdim=False, uops_sha={ver: s.sha(ver)})
    dvo.OPS.append(op)
    dvo.CUSTOM_DVE_SPECS[name] = spec
    dvo._SUB_OPCODE_FOR_NAME[name] = row
    return op


def _t1_ref(in0, in1, c0, c1, c2):
    x = in0.astype(np.float32)
    u = np.square(x)
    return x * (c0 + u * (c1 + u * c2))


_u = dsq(Src0)
T1C = _register_op("BILAT_T1_ANT", Src0 * (C0 + _u * (C1 + _u * C2)), _t1_ref,
                   rd1=False)


def _rden_ref(in0, in1, c0, c1, c2):
    x = in0.astype(np.float32) + in1.astype(np.float32)
    y0 = c0 - c1 * x
    return y0 * (2.0 - x * y0)


_x = Src0 + Src1
_y0 = C0 - C1 * _x
_e = _y0 * (One - _x * _y0)
RDEN = _register_op("BILAT_RDEN_ANT", _e + _y0, _rden_ref, rd1=True)

# ---- weights ---------------------------------------------------------------

_W1D = np.exp(-0.5 * np.array([4.0, 1.0, 0.0, 1.0, 4.0], dtype=np.float64)).astype(
    np.float32
)


def _e4m3(a):
    return np.asarray(a, np.float32).astype(NP_F8).astype(np.float32)


def _build_bands8() -> np.ndarray:
    """b8[q, dy, kt, o]: e4m3 band + e4m3 residual in the second k-tile."""
    b = np.zeros((CH_IN, 5, 2, M8), dtype=np.float32)
    for o in range(CH_OUT):
        for d in range(5):
            for dy in range(5):
                w = np.float32(_W1D[d] * _W1D[dy])
                w0 = _e4m3(w)
                b[o + d, dy, 0, o] = w0
                b[o + d, dy, 1, o] = _e4m3(w - w0)
    return b.astype(NP_F8)


# engine-assignment tunables
T2_ON_POOL = True    # t2 = t1*x on Pool (else DVE)
EM_ON_POOL = True    # em = s1e + qm on Pool (else DVE)
SQ_ON_ACT = True     # sq via ACT Square (else DVE TT)
WARM_MM = 15         # PE p-state warmup matmuls before real work


def build_nc(degree: int = DEGREE, bench_iters: int = 1):
    nc = bacc.Bacc("TRN2", target_bir_lowering=False)
    const_tensors = []
    for v in (0.125, 0.0):
        t_ = nc.alloc_sbuf_tensor(f"const-f32-{v}", [128, 1], F32)
        nc.const_aps.aps[(F32, v)] = t_.ap()
        const_tensors.append((t_, v))
    x_d = nc.dram_tensor("x", [WPAD, NIMG, R], F16, kind="ExternalInput")
    b8_d = nc.dram_tensor("b8", [CH_IN, 5, 2, M8], F8, kind="ExternalInput")
    y_d = nc.dram_tensor("y", [WPAD, 2, NMOV], F16, kind="ExternalOutput")

    with ExitStack() as ctx:
        tc = ctx.enter_context(tile.TileContext(nc))
        singles = ctx.enter_context(tc.tile_pool(name="singles", bufs=1))
        fields = ctx.enter_context(tc.tile_pool(name="fields", bufs=3))
        asm = ctx.enter_context(tc.tile_pool(name="asm", bufs=3))
        psum = ctx.enter_context(tc.tile_pool(name="psum", bufs=1, space="PSUM"))

        for t_, v in const_tensors:
            nc.gpsimd.memset(t_.ap(), v)
        b8 = singles.tile([CH_IN, 5, 2, M8], F8)
        # dummy activation: hoists the ACT table load into the DMA window
        warm = singles.tile([128, 1], F16)
        nc.scalar.activation(out=warm, in_=const_tensors[0][0].ap(),
                             func=AF.Exp, bias=0.125, scale=-0.5)
        # PE p-state warmup: the tensor engine ramps to full clock only after
        # ~3us of continuous execution; burn idle pre-DMA time on dummy
        # matmuls over memset data so real convs run at full speed.
        wmov = singles.tile([CH_IN, NIMG, R], F8)
        wst = singles.tile([CH_IN, 2, M8], F8)
        nc.gpsimd.memset(wmov[:], 0.0)
        nc.gpsimd.memset(wst[:], 0.0)

        def mov8(t, g, dy):
            """[108, 2(step 0), 404] moving AP into field tile t at group g, dy."""
            full = t[:]
            ap0 = [list(d) for d in full.ap][0]
            off = full.offset + g * (GRP * R) + dy
            return AP(full.tensor, off, [ap0, [0, 2], [1, NMOV]])

        def xc_src(j):
            """DRAM view [104, 2(g), 404] of centered x for chunk j's centers."""
            c0 = CH_OUT * j
            sl = x_d[c0 + 2 : c0 + 2 + CH_OUT]
            pdim = [list(d) for d in sl.ap][0]
            return AP(sl.tensor, sl.offset + 2,
                      [pdim, [GRP * R, 2], [1, NMOV]])

        def bc2(t):
            """[104, 2(bcast), 2(g), 404] view of a [104, 2, 404] tile."""
            full = t[:]
            pdim = [list(d) for d in full.ap][0]
            return AP(full.tensor, full.offset,
                      [pdim, [0, 2], [NMOV, 2], [1, NMOV]])

        def body():
            xts, xcs, flds = {}, {}, {}

            def load_x(j):
                c0 = CH_OUT * j
                x_t = fields.tile([CH_IN, NIMG, R], F16, name="x_t", tag="x_t",
                                  bufs=NCHUNK)
                nc.sync.dma_start(out=x_t, in_=x_d[c0 : c0 + CH_IN])
                xts[j] = x_t

            def load_xc(j):
                x_cg = fields.tile([CH_OUT, 2, NMOV], F16, name="x_cg",
                                   tag="x_cg", bufs=NCHUNK)
                nc.sync.dma_start(out=x_cg, in_=xc_src(j))
                xcs[j] = x_cg

            def load_fields(j):
                x_t = xts[j]
                sqt = fields.tile([CH_IN, NIMG, R], F16, name="sq", tag="sq",
                                  bufs=2)
                if SQ_ON_ACT:
                    nc.scalar.activation(out=sqt, in_=x_t, func=AF.Square,
                                         bias=0.0, scale=1.0)
                else:
                    nc.vector.tensor_mul(sqt, x_t, x_t)
                t0 = fields.tile([CH_IN, NIMG, R], F8, name="t0", tag="t0",
                                 bufs=2)
                nc.scalar.activation(out=t0, in_=sqt, func=AF.Exp,
                                     bias=0.125, scale=-0.5)
                t1 = fields.tile([CH_IN, NIMG, R], F8, name="t1", tag="t1",
                                 bufs=2)
                nc.vector._custom_dve(T1C, out=t1[:], in0=x_t[:],
                                      s0=_P0, s1=_P1, imm2=_P2)
                t2 = fields.tile([CH_IN, NIMG, R], F8, name="t2", tag="t2",
                                 bufs=2)
                eng = nc.gpsimd if T2_ON_POOL else nc.vector
                eng.tensor_mul(t2, t1, x_t)
                flds[j] = (t0, t1, t2)

            def conv(ps_slice, tk, g):
                for dy in range(5):
                    nc.tensor.matmul(
                        ps_slice,
                        b8[:, dy, :, :],
                        mov8(tk, g, dy),
                        start=(dy == 0),
                        stop=(dy == 4),
                        perf_mode=mybir.MatmulPerfMode.DoubleRow,
                    )

            def warmup(n_mm):
                """Dummy DR matmuls to ramp PE; borrows a ps02 buffer (PE-serial)."""
                wps = psum.tile([M8, 2, 512], F32, name="wps", tag="ps02",
                                bufs=2)
                for i in range(n_mm):
                    nc.tensor.matmul(
                        wps[:, 0, 0:NMOV], wst[:],
                        AP(wmov[:].tensor, wmov[:].offset,
                           [[list(d) for d in wmov[:].ap][0], [0, 2], [1, NMOV]]),
                        start=True, stop=True,
                        perf_mode=mybir.MatmulPerfMode.DoubleRow,
                    )

            def chunk_stage(j):
                """Convs + evac + asm for chunk j, ordered S1, S0, S2 to match
                field readiness (t1 DVE first, t0 ACT, t2 Pool last); evacs
                overlap later convs and free psum banks early."""
                t0, t1, t2 = flds[j]
                c0 = CH_OUT * j
                n_out = min(CH_OUT, W - c0)
                # ps1 double-buffered: S1(j+1) must not wait on the s1e evac.
                # ps0/ps2 share one double-buffered tag (alternating gens), so
                # each still gets a full chunk of slack. 4+4 = 8 banks.
                ps0 = psum.tile([M8, 2, 512], F32, name="ps0", tag="ps02",
                                bufs=2)
                ps1 = psum.tile([M8, 2, 512], F32, name="ps1", tag="ps1",
                                bufs=2)
                ps2 = psum.tile([M8, 2, 512], F32, name="ps2", tag="ps02",
                                bufs=2)
                s12e = asm.tile([CH_OUT, 2, 2, NMOV], F16, name="s12e",
                                tag="s12e")
                last = j == NCHUNK - 1
                for g in range(2):
                    conv(ps1[:, g, 0:NMOV], t1, g)
                nc.scalar.copy(out=s12e[:, 0], in_=ps1[0:CH_OUT, :, 0:NMOV])
                # qd = x_c * S1 early, so rden can overlap later matmuls
                qd = asm.tile([CH_OUT, 2, NMOV], F16, name="qd", tag="qd")
                nc.vector.tensor_mul(qd, xcs[j], s12e[:, 0])
                rden = asm.tile([CH_OUT, 2, NMOV], F16, name="rden", tag="rden")
                qm = asm.tile([CH_OUT, 2, NMOV], F16, name="qm", tag="qm")
                em = asm.tile([CH_OUT, 2, NMOV], F16, name="em", tag="em")

                def do_rden():
                    # rden = 1/(S0 + qd), S0 read straight from PSUM
                    nc.vector._custom_dve(
                        RDEN, out=rden[:], in0=ps0[0:CH_OUT, :, 0:NMOV],
                        in1=qd[:], s0=_NA, s1=_NB,
                    )

                def do_s2_chain():
                    nc.scalar.copy(out=s12e[:, 1],
                                   in_=ps2[0:CH_OUT, :, 0:NMOV])
                    nc.vector.tensor_mul(qm, xcs[j], s12e[:, 1])
                    # em = S1 + qm; DVE for the tail chunks (Pool's 1.7us
                    # ops would serialize the drain after PE finishes)
                    eng = nc.gpsimd if (EM_ON_POOL and j < 3) else nc.vector
                    eng.tensor_add(em, s12e[:, 0], qm)

                if last:
                    # S2 before S0: the s2e/qm/em chain overlaps the S0
                    # matmuls, leaving only rden+outm as post-PE tail.
                    for g in range(2):
                        conv(ps2[:, g, 0:NMOV], t2, g)
                    do_s2_chain()
                    for g in range(2):
                        conv(ps0[:, g, 0:NMOV], t0, g)
                    do_rden()
                else:
                    for g in range(2):
                        conv(ps0[:, g, 0:NMOV], t0, g)
                    do_rden()
                    for g in range(2):
                        conv(ps2[:, g, 0:NMOV], t2, g)
                    do_s2_chain()
                if j + 2 < NCHUNK:
                    load_fields(j + 2)
                # outm = em * rden
                outm = asm.tile([CH_OUT, 2, NMOV], F16, name="outm", tag="outm")
                nc.vector.tensor_mul(outm, em, rden)
                nc.sync.dma_start(
                    out=y_d[c0 + 2 : c0 + 2 + n_out], in_=outm[:n_out]
                )

            warmup(WARM_MM)
            load_x(0)
            nc.sync.dma_start(out=b8, in_=b8_d[:])
            load_x(1)
            load_x(2)
            load_xc(0)
            load_x(3)
            load_xc(1)
            load_x(4)
            load_xc(2)
            load_xc(3)
            load_xc(4)
            load_fields(0)
            load_fields(1)
            for j in range(NCHUNK):
                chunk_stage(j)

        if bench_iters == 1:
            body()
        else:
            hints = (
                mybir.EngineType.PE,
                mybir.EngineType.DVE,
                mybir.EngineType.Activation,
                mybir.EngineType.SP,
            )
            with tc.For_i(0, bench_iters, 1, hint_engines=hints):
                body()

    nc.finalize()
    return nc


def _prep_inputs(X: np.ndarray):
    """Full X [4,3,512,512] fp32 -> per-core centered/transposed fp16 + weights."""
    Xr = np.ascontiguousarray(np.asarray(X, dtype=np.float32).reshape(NIMG, H, W))
    b8 = _build_bands8()
    in_maps = []
    for i in range(N_CORES):
        lo = ROWS * i - 2
        s0, s1 = max(0, lo), min(H, lo + R)
        # reference zero-pads in p-space; centered x = p - 1/2 -> pad is -0.5
        P = np.full((NIMG, R, WPAD), -0.5, dtype=np.float32)
        P[:, s0 - lo : s1 - lo, 2 : 2 + W] = Xr[:, s0:s1, :] - 0.5
        xt = np.ascontiguousarray(P.transpose(2, 0, 1)).astype(NP_F16)
        in_maps.append({"x": xt, "b8": b8})
    return in_maps


_NC_CACHE = {}

# host-side gather index: flat m -> (img, row) keeping img-local rows < 64
_MVALID = np.array([m for m in range(NMOV) if m % R < ROWS], dtype=np.int64)


def kernel(X: np.ndarray) -> np.ndarray:
    key = (DEGREE, 1)
    if key not in _NC_CACHE:
        _NC_CACHE[key] = build_nc(DEGREE, 1)
    nc = _NC_CACHE[key]
    in_maps = _prep_inputs(X)
    res = run_bass_kernel_spmd(nc, in_maps, list(range(N_CORES)))
    out = np.empty((NIMG, H, W), dtype=np.float32)
    for i in range(N_CORES):
        yi = np.asarray(res.results[i]["y"], dtype=np.float32)  # [WPAD, 2, NMOV]
        # [W, 2, 384] -> (g, img-in-grp, row) -> [12, 64, W]
        yv = yi[2 : 2 + W][:, :, _MVALID].reshape(W, 2, GRP, ROWS)
        out[:, ROWS * i : ROWS * (i + 1), :] = (
            0.5 + yv.reshape(W, NIMG, ROWS).transpose(1, 2, 0)
        )
    return out.reshape(4, 3, H, W)
